# revision 3
# baseline (speedup 1.0000x reference)
"""Trainium2 Bass kernel for the 4-layer spiking-MLP critic (T=16 IF/LIF recurrence).

Strategy
- Data-parallel over 8 NeuronCores: batch 4096 -> 512 per core; weights replicated.
- Everything runs transposed (feature dim on partitions, batch on the free dim),
  so no on-device transposes are needed anywhere.
- x @ W1.T + b1 is time-invariant: computed once into SBUF, reused all 16 steps.
- Weights and spikes are fp16; full fp32 accuracy is recovered with a hi/lo
  split: W ~= Whi + 2^-11 * Wlo (both fp16). Spikes are 0/1 (exact in fp16), so
  each layer is two fp16 matmul groups; the lo PSUM is folded in with a single
  fused scalar_tensor_tensor op ((lo * 2^-11) + hi).
- Layer-4 (non-spiking LIF, tau=2) is algebraically unrolled:
      v4_T = 2^-16 * sum_t 2^t * (s3_t @ W4.T) + (1 - 2^-16) * b4
  The weighted sum accumulates directly in a persistent PSUM bank across all 16
  steps by scaling the spike tensor with 2^t (exact in fp16), eliminating all
  per-step layer-4 elementwise work and state.
- IF membrane states carry their bias folded in (vb = v + b), saving one
  elementwise op per layer per step.
"""

import sys

sys.path.insert(0, "/opt/trn_rl_repo")

import numpy as np

P = 128
D, H, AOUT = 512, 1024, 64
N = 512           # batch per core
T = 16
KD, KH = D // P, H // P
CLO = float(2.0 ** -11)
NCORES = 8

_CACHE = {}


def _build():
    from contextlib import ExitStack
    from concourse import bacc, mybir, tile

    f32 = mybir.dt.float32
    f16 = mybir.dt.float16
    A = mybir.AluOpType
    IDENT = mybir.ActivationFunctionType.Identity

    nc = bacc.Bacc("TRN2", target_bir_lowering=False, debug=False)

    din = {}
    for name, shape, dt_ in [
        ("xh", [D, N], f16), ("xl", [D, N], f16),
        ("w1h", [D, H], f16), ("w1l", [D, H], f16),
        ("w2h", [H, H], f16), ("w2l", [H, H], f16),
        ("w3h", [H, H], f16), ("w3l", [H, H], f16),
        ("w4h", [H, AOUT], f16), ("w4l", [H, AOUT], f16),
        ("b1", [P, KH], f32), ("b2", [P, KH], f32), ("b3", [P, KH], f32),
        ("b4f", [AOUT, 1], f32),
    ]:
        din[name] = nc.dram_tensor(name, shape, dt_, kind="ExternalInput")
    dout = nc.dram_tensor("v4T", [AOUT, N], f32, kind="ExternalOutput")

    ts = lambda i, sz: slice(i * sz, (i + 1) * sz)

    with tile.TileContext(nc) as tc, ExitStack() as ctx:
        wpool = ctx.enter_context(tc.tile_pool(name="w", bufs=1))
        vpool = ctx.enter_context(tc.tile_pool(name="v", bufs=1))
        spool = ctx.enter_context(tc.tile_pool(name="s", bufs=1))
        upool = ctx.enter_context(tc.tile_pool(name="u", bufs=2))
        tpool = ctx.enter_context(tc.tile_pool(name="t", bufs=3))
        npool = ctx.enter_context(tc.tile_pool(name="n", bufs=2))
        mmps = ctx.enter_context(tc.tile_pool(name="mmps", bufs=2, space="PSUM"))
        zps = ctx.enter_context(tc.tile_pool(name="zps", bufs=1, space="PSUM"))

        def load_km(name, ko, m):
            t_ = wpool.tile([P, ko, m], f16, tag=name)
            nc.sync.dma_start(t_[:], din[name].ap().rearrange("(ko p) m -> p ko m", p=P))
            return t_

        w2h, w2l = load_km("w2h", KH, H), load_km("w2l", KH, H)
        w3h, w3l = load_km("w3h", KH, H), load_km("w3l", KH, H)
        w4h, w4l = load_km("w4h", KH, AOUT), load_km("w4l", KH, AOUT)

        b1sb = wpool.tile([P, KH], f32, tag="b1")
        nc.sync.dma_start(b1sb[:], din["b1"].ap())
        b2sb = wpool.tile([P, KH], f32, tag="b2")
        nc.sync.dma_start(b2sb[:], din["b2"].ap())
        b3sb = wpool.tile([P, KH], f32, tag="b3")
        nc.sync.dma_start(b3sb[:], din["b3"].ap())
        b4sb = wpool.tile([AOUT, 1], f32, tag="b4f")
        nc.sync.dma_start(b4sb[:], din["b4f"].ap())

        dv1 = vpool.tile([P, KH, N], f32, tag="dv1")
        v1 = vpool.tile([P, KH, N], f32, tag="v1")
        vb2 = vpool.tile([P, KH, N], f32, tag="vb2")
        vb3 = vpool.tile([P, KH, N], f32, tag="vb3")
        s1 = spool.tile([P, KH, N], f16, tag="s1")
        s2 = spool.tile([P, KH, N], f16, tag="s2")
        s3 = spool.tile([P, KH, N], f16, tag="s3")

        nc.gpsimd.memset(v1[:], 0.0)
        nc.gpsimd.memset(vb2[:], 0.0)
        nc.gpsimd.memset(vb3[:], 0.0)
        for c in range(KH):
            nc.scalar.activation(vb2[:, c, :], vb2[:, c, :], IDENT, bias=b2sb[:, ts(c, 1)])
            nc.scalar.activation(vb3[:, c, :], vb3[:, c, :], IDENT, bias=b3sb[:, ts(c, 1)])

        zh = zps.tile([AOUT, N], f32, tag="zh")
        zl = zps.tile([AOUT, N], f32, tag="zl")

        # ---- dv1 = x @ W1.T + b1, in hi/lo pieces (x itself is split too) ----
        with tc.tile_pool(name="startup", bufs=1) as stp:
            xh = stp.tile([P, KD, N], f16, tag="xh")
            nc.sync.dma_start(xh[:], din["xh"].ap().rearrange("(ko p) m -> p ko m", p=P))
            xl = stp.tile([P, KD, N], f16, tag="xl")
            nc.sync.dma_start(xl[:], din["xl"].ap().rearrange("(ko p) m -> p ko m", p=P))
            w1h = stp.tile([P, KD, H], f16, tag="w1h")
            nc.sync.dma_start(w1h[:], din["w1h"].ap().rearrange("(ko p) m -> p ko m", p=P))
            w1l = stp.tile([P, KD, H], f16, tag="w1l")
            nc.sync.dma_start(w1l[:], din["w1l"].ap().rearrange("(ko p) m -> p ko m", p=P))

            for c in range(KH):
                ph = mmps.tile([P, N], f32, tag="ph")
                pl = mmps.tile([P, N], f32, tag="pl")
                for k in range(KD):
                    nc.tensor.matmul(ph[:], w1h[:, k, ts(c, P)], xh[:, k, :],
                                     start=(k == 0), stop=(k == KD - 1))
                for i, (wt, xt) in enumerate([(w1l, xh), (w1h, xl)]):
                    for k in range(KD):
                        nc.tensor.matmul(pl[:], wt[:, k, ts(c, P)], xt[:, k, :],
                                         start=(i == 0 and k == 0),
                                         stop=(i == 1 and k == KD - 1))
                tt = tpool.tile([P, N], f32, tag="t")
                nc.vector.tensor_scalar(tt[:], pl[:], CLO, None, A.mult)
                hh = tpool.tile([P, N], f32, tag="t")
                nc.scalar.activation(hh[:], ph[:], IDENT, bias=b1sb[:, ts(c, 1)])
                nc.vector.tensor_tensor(dv1[:, c, :], hh[:], tt[:], A.add)

        # ---- the 16-step recurrence ----
        def if_layer(s_in, wh, wl, vb, bsb, s_out, pow2):
            for c in range(KH):
                ph = mmps.tile([P, N], f32, tag="ph")
                pl = mmps.tile([P, N], f32, tag="pl")
                for k in range(KH):
                    nc.tensor.matmul(ph[:], wh[:, k, ts(c, P)], s_in[:, k, :],
                                     start=(k == 0), stop=(k == KH - 1))
                for k in range(KH):
                    nc.tensor.matmul(pl[:], wl[:, k, ts(c, P)], s_in[:, k, :],
                                     start=(k == 0), stop=(k == KH - 1))
                tt = tpool.tile([P, N], f32, tag="t")
                nc.vector.scalar_tensor_tensor(tt[:], pl[:], CLO, vb[:, c, :], A.mult, A.add)
                u = upool.tile([P, N], f32, tag="u")
                nc.vector.tensor_tensor(u[:], ph[:], tt[:], A.add)
                if pow2 is None:
                    nc.gpsimd.tensor_scalar(s_out[:, c, :], u[:], 1.0, None, A.is_ge)
                else:
                    nc.vector.tensor_scalar(s_out[:, c, :], u[:], 1.0, pow2, A.is_ge, A.mult)
                nn = npool.tile([P, N], f16, tag="n")
                nc.gpsimd.tensor_scalar(nn[:], u[:], 1.0, None, A.is_lt)
                t2 = tpool.tile([P, N], f32, tag="t")
                nc.vector.tensor_tensor(t2[:], u[:], nn[:], A.mult)
                nc.scalar.activation(vb[:, c, :], t2[:], IDENT, bias=bsb[:, ts(c, 1)])

        for t in range(T):
            # layer 1: dv1 is constant; pure elementwise
            for c in range(KH):
                u = upool.tile([P, N], f32, tag="u")
                nc.vector.tensor_tensor(u[:], dv1[:, c, :], v1[:, c, :], A.add)
                nc.gpsimd.tensor_scalar(s1[:, c, :], u[:], 1.0, None, A.is_ge)
                nn = npool.tile([P, N], f16, tag="n")
                nc.gpsimd.tensor_scalar(nn[:], u[:], 1.0, None, A.is_lt)
                nc.vector.tensor_tensor(v1[:, c, :], u[:], nn[:], A.mult)

            if_layer(s1, w2h, w2l, vb2, b2sb, s2, None)
            if_layer(s2, w3h, w3l, vb3, b3sb, s3, float(2.0 ** t))

            for k in range(KH):
                nc.tensor.matmul(zh[:], w4h[:, k, :], s3[:, k, :],
                                 start=(t == 0 and k == 0), stop=(t == T - 1 and k == KH - 1),
                                 skip_group_check=True)
            for k in range(KH):
                nc.tensor.matmul(zl[:], w4l[:, k, :], s3[:, k, :],
                                 start=(t == 0 and k == 0), stop=(t == T - 1 and k == KH - 1),
                                 skip_group_check=True)

        fl = tpool.tile([AOUT, N], f32, tag="fin")
        nc.vector.tensor_scalar(fl[:], zl[:], CLO, None, A.mult)
        fin = tpool.tile([AOUT, N], f32, tag="fin")
        nc.vector.tensor_tensor(fin[:], zh[:], fl[:], A.add)
        fout = tpool.tile([AOUT, N], f32, tag="fout")
        nc.scalar.activation(fout[:], fin[:], IDENT, scale=float(2.0 ** -T), bias=b4sb[:])
        nc.sync.dma_start(dout.ap(), fout[:])

    nc.compile()
    return nc


def _hilo(a):
    hi = a.astype(np.float16)
    lo = ((a.astype(np.float32) - hi.astype(np.float32)) * np.float32(2.0 ** 11)).astype(np.float16)
    return hi, lo


def _prep_inputs(x, W1, b1, W2, b2, W3, b3, W4, b4):
    xT = np.ascontiguousarray(x.T.astype(np.float32))          # (D, B)
    xh, xl = _hilo(xT)
    w1h, w1l = _hilo(np.ascontiguousarray(W1.T))               # (D, H)
    w2h, w2l = _hilo(np.ascontiguousarray(W2.T))               # (H, H)
    w3h, w3l = _hilo(np.ascontiguousarray(W3.T))
    w4h, w4l = _hilo(np.ascontiguousarray(W4.T))               # (H, AOUT)
    shared = {
        "w1h": w1h, "w1l": w1l, "w2h": w2h, "w2l": w2l,
        "w3h": w3h, "w3l": w3l, "w4h": w4h, "w4l": w4l,
        "b1": np.ascontiguousarray(b1.reshape(KH, P).T.astype(np.float32)),
        "b2": np.ascontiguousarray(b2.reshape(KH, P).T.astype(np.float32)),
        "b3": np.ascontiguousarray(b3.reshape(KH, P).T.astype(np.float32)),
        "b4f": ((1.0 - 2.0 ** -T) * b4).astype(np.float32).reshape(AOUT, 1),
    }
    in_maps = []
    for i in range(NCORES):
        m = dict(shared)
        m["xh"] = np.ascontiguousarray(xh[:, i * N:(i + 1) * N])
        m["xl"] = np.ascontiguousarray(xl[:, i * N:(i + 1) * N])
        in_maps.append(m)
    return in_maps


def _run(in_maps):
    from concourse.bass_utils import run_bass_kernel_spmd
    if "nc" not in _CACHE:
        _CACHE["nc"] = _build()
    res = run_bass_kernel_spmd(_CACHE["nc"], in_maps, list(range(NCORES)))
    parts = [res.results[i]["v4T"] for i in range(NCORES)]     # each (AOUT, N)
    return np.ascontiguousarray(np.concatenate(parts, axis=1).T).astype(np.float32)


def kernel(x, W1, b1, W2, b2, W3, b3, W4, b4):
    in_maps = _prep_inputs(x, W1, b1, W2, b2, W3, b3, W4, b4)
    return _run(in_maps)


# revision 8
# speedup vs baseline: 1.1173x; 1.1173x over previous
"""Trainium2 Bass kernel for the 4-layer spiking-MLP critic (T=16 IF/LIF recurrence).

Strategy
- Data-parallel over 8 NeuronCores: batch 4096 -> 512 per core; weights replicated.
- Everything runs transposed (feature dim on partitions, batch on the free dim),
  so no on-device transposes are needed anywhere.
- x @ W1.T + b1 is time-invariant: computed once into SBUF, reused all 16 steps.
- Weights and spikes are fp16; full fp32 accuracy is recovered with a hi/lo
  split: W ~= Whi + 2^-11 * Wlo (both fp16). Spikes are 0/1 (exact in fp16), so
  each layer is two fp16 matmul groups; the lo PSUM is folded in with a single
  fused scalar_tensor_tensor op ((lo * 2^-11) + hi).
- Layer-4 (non-spiking LIF, tau=2) is algebraically unrolled:
      v4_T = 2^-16 * sum_t 2^t * (s3_t @ W4.T) + (1 - 2^-16) * b4
  The weighted sum accumulates directly in a persistent PSUM bank across all 16
  steps by scaling the spike tensor with 2^t (exact in fp16), eliminating all
  per-step layer-4 elementwise work and state.
- IF membrane states carry their bias folded in (vb = v + b), saving one
  elementwise op per layer per step.
"""

import sys

sys.path.insert(0, "/opt/trn_rl_repo")

import numpy as np

P = 128
D, H, AOUT = 512, 1024, 64
N = 512           # batch per core
import os
T = int(os.environ.get("KERNEL_T", "16"))
KD, KH = D // P, H // P
CLO = float(2.0 ** -11)
NCORES = 8

_CACHE = {}


def _build():
    from contextlib import ExitStack
    from concourse import bacc, mybir, tile

    f32 = mybir.dt.float32
    f16 = mybir.dt.float16
    A = mybir.AluOpType
    IDENT = mybir.ActivationFunctionType.Identity

    nc = bacc.Bacc("TRN2", target_bir_lowering=False, debug=False)

    din = {}
    for name, shape, dt_ in [
        ("xh", [D, N], f16), ("xl", [D, N], f16),
        ("w1h", [D, H], f16), ("w1l", [D, H], f16),
        ("w2h", [H, H], f16), ("w2l", [H, H], f16),
        ("w3h", [H, H], f16), ("w3l", [H, H], f16),
        ("w4h", [H, AOUT], f16), ("w4l", [H, AOUT], f16),
        ("b1", [P, KH], f32), ("b2", [P, KH], f32), ("b3", [P, KH], f32),
        ("b4f", [AOUT, 1], f32),
    ]:
        din[name] = nc.dram_tensor(name, shape, dt_, kind="ExternalInput")
    dout = nc.dram_tensor("v4T", [AOUT, N], f32, kind="ExternalOutput")

    ts = lambda i, sz: slice(i * sz, (i + 1) * sz)

    with tile.TileContext(nc) as tc, ExitStack() as ctx:
        wpool = ctx.enter_context(tc.tile_pool(name="w", bufs=1))
        vpool = ctx.enter_context(tc.tile_pool(name="v", bufs=1))
        spool = ctx.enter_context(tc.tile_pool(name="s", bufs=1))
        upool = ctx.enter_context(tc.tile_pool(name="u", bufs=3))
        tpool = ctx.enter_context(tc.tile_pool(name="t", bufs=3))
        npool = ctx.enter_context(tc.tile_pool(name="n", bufs=2))
        mmps = ctx.enter_context(tc.tile_pool(name="mmps", bufs=3, space="PSUM"))
        zps = ctx.enter_context(tc.tile_pool(name="zps", bufs=1, space="PSUM"))

        def load_km(name, ko, m):
            t_ = wpool.tile([P, ko, m], f16, tag=name)
            nc.sync.dma_start(t_[:], din[name].ap().rearrange("(ko p) m -> p ko m", p=P))
            return t_

        w2h, w2l = load_km("w2h", KH, H), load_km("w2l", KH, H)
        w3h, w3l = load_km("w3h", KH, H), load_km("w3l", KH, H)
        w4h = load_km("w4h", KH, AOUT)

        b1sb = wpool.tile([P, KH], f32, tag="b1")
        nc.sync.dma_start(b1sb[:], din["b1"].ap())
        b2sb = wpool.tile([P, KH], f32, tag="b2")
        nc.sync.dma_start(b2sb[:], din["b2"].ap())
        b3sb = wpool.tile([P, KH], f32, tag="b3")
        nc.sync.dma_start(b3sb[:], din["b3"].ap())
        b4sb = wpool.tile([AOUT, 1], f32, tag="b4f")
        nc.sync.dma_start(b4sb[:], din["b4f"].ap())

        dv1 = vpool.tile([P, KH, N], f32, tag="dv1")
        v1 = vpool.tile([P, KH, N], f32, tag="v1")
        vb2 = vpool.tile([P, KH, N], f32, tag="vb2")
        vb3 = vpool.tile([P, KH, N], f32, tag="vb3")
        s1 = spool.tile([P, KH, N], f16, tag="s1")
        s2 = spool.tile([P, KH, N], f16, tag="s2")
        s3 = spool.tile([P, KH, N], f16, tag="s3")

        nc.gpsimd.memset(v1[:], 0.0)
        nc.gpsimd.memset(vb2[:], 0.0)
        nc.gpsimd.memset(vb3[:], 0.0)
        for c in range(KH):
            nc.scalar.activation(vb2[:, c, :], vb2[:, c, :], IDENT, bias=b2sb[:, ts(c, 1)])
            nc.scalar.activation(vb3[:, c, :], vb3[:, c, :], IDENT, bias=b3sb[:, ts(c, 1)])

        zh = zps.tile([AOUT, N], f32, tag="zh")

        # ---- dv1 = x @ W1.T + b1, in hi/lo pieces (x itself is split too) ----
        def _make_dv1_half(stp, xh, xl):
            def _dv1_half(half, w1h, w1l):
                for cc in range(KH // 2):
                    c = half * (KH // 2) + cc
                    ph = mmps.tile([P, N], f32, tag="ph")
                    pl = mmps.tile([P, N], f32, tag="pl")
                    for k in range(KD):
                        nc.tensor.matmul(ph[:], w1h[:, k, ts(cc, P)], xh[:, k, :],
                                         start=(k == 0), stop=(k == KD - 1))
                    for i, (wt, xt) in enumerate([(w1l, xh), (w1h, xl)]):
                        for k in range(KD):
                            nc.tensor.matmul(pl[:], wt[:, k, ts(cc, P)], xt[:, k, :],
                                             start=(i == 0 and k == 0),
                                             stop=(i == 1 and k == KD - 1))
                    tt = tpool.tile([P, N], f32, tag="t")
                    nc.vector.tensor_scalar(tt[:], pl[:], CLO, None, A.mult)
                    hh = tpool.tile([P, N], f32, tag="t")
                    nc.scalar.activation(hh[:], ph[:], IDENT, bias=b1sb[:, ts(c, 1)])
                    nc.vector.tensor_tensor(dv1[:, c, :], hh[:], tt[:], A.add)
            return _dv1_half

        with tc.tile_pool(name="startup", bufs=1) as stp:
            xh = stp.tile([P, KD, N], f16, tag="xh")
            nc.sync.dma_start(xh[:], din["xh"].ap().rearrange("(ko p) m -> p ko m", p=P))
            xl = stp.tile([P, KD, N], f16, tag="xl")
            nc.sync.dma_start(xl[:], din["xl"].ap().rearrange("(ko p) m -> p ko m", p=P))
            _dv1_half = _make_dv1_half(stp, xh, xl)
            for half in range(2):
                w1h = stp.tile([P, KD, H // 2], f16, tag="w1h")
                nc.sync.dma_start(
                    w1h[:], din["w1h"].ap().rearrange("(ko p) m -> p ko m", p=P)[:, :, ts(half, H // 2)])
                w1l = stp.tile([P, KD, H // 2], f16, tag="w1l")
                nc.sync.dma_start(
                    w1l[:], din["w1l"].ap().rearrange("(ko p) m -> p ko m", p=P)[:, :, ts(half, H // 2)])
                _dv1_half(half, w1h, w1l)

        # ---- the 16-step recurrence ----
        def if_layer(s_in, wh, wl, vb, bsb, s_out, pow2):
            for c in range(KH):
                ph = mmps.tile([P, N], f32, tag="ph")
                pl = mmps.tile([P, N], f32, tag="pl")
                for k in range(KH):
                    nc.tensor.matmul(pl[:], wl[:, k, ts(c, P)], s_in[:, k, :],
                                     start=(k == 0), stop=(k == KH - 1))
                for k in range(KH):
                    nc.tensor.matmul(ph[:], wh[:, k, ts(c, P)], s_in[:, k, :],
                                     start=(k == 0), stop=(k == KH - 1))
                tt = tpool.tile([P, N], f32, tag="t")
                nc.vector.scalar_tensor_tensor(tt[:], pl[:], CLO, vb[:, c, :], A.mult, A.add)
                u = upool.tile([P, N], f32, tag="u")
                nc.vector.tensor_tensor(u[:], ph[:], tt[:], A.add)
                if pow2 is None:
                    nc.gpsimd.tensor_scalar(s_out[:, c, :], u[:], 1.0, None, A.is_ge)
                else:
                    nc.vector.tensor_scalar(s_out[:, c, :], u[:], 1.0, pow2, A.is_ge, A.mult)
                nn = npool.tile([P, N], f16, tag="n")
                nc.gpsimd.tensor_scalar(nn[:], u[:], 1.0, None, A.is_lt)
                t2 = tpool.tile([P, N], f32, tag="t2")
                nc.gpsimd.tensor_tensor(t2[:], u[:], nn[:], A.mult)
                nc.scalar.activation(vb[:, c, :], t2[:], IDENT, bias=bsb[:, ts(c, 1)])

        for t in range(T):
            # layer 1: dv1 is constant; pure elementwise
            for c in range(KH):
                u = upool.tile([P, N], f32, tag="u")
                nc.vector.tensor_tensor(u[:], dv1[:, c, :], v1[:, c, :], A.add)
                nc.gpsimd.tensor_scalar(s1[:, c, :], u[:], 1.0, None, A.is_ge)
                nn = npool.tile([P, N], f16, tag="n")
                nc.gpsimd.tensor_scalar(nn[:], u[:], 1.0, None, A.is_lt)
                nc.vector.tensor_tensor(v1[:, c, :], u[:], nn[:], A.mult)

            if_layer(s1, w2h, w2l, vb2, b2sb, s2, None)
            if_layer(s2, w3h, w3l, vb3, b3sb, s3, float(2.0 ** t))

            for k in range(KH):
                nc.tensor.matmul(zh[:], w4h[:, k, :], s3[:, k, :],
                                 start=(t == 0 and k == 0), stop=(t == T - 1 and k == KH - 1),
                                 skip_group_check=True)

        fout = tpool.tile([AOUT, N], f32, tag="fout")
        nc.scalar.activation(fout[:], zh[:], IDENT, scale=float(2.0 ** -T), bias=b4sb[:])
        nc.sync.dma_start(dout.ap(), fout[:])

    nc.compile()
    return nc


def _hilo(a):
    hi = a.astype(np.float16)
    lo = ((a.astype(np.float32) - hi.astype(np.float32)) * np.float32(2.0 ** 11)).astype(np.float16)
    return hi, lo


def _prep_inputs(x, W1, b1, W2, b2, W3, b3, W4, b4):
    xT = np.ascontiguousarray(x.T.astype(np.float32))          # (D, B)
    xh, xl = _hilo(xT)
    w1h, w1l = _hilo(np.ascontiguousarray(W1.T))               # (D, H)
    w2h, w2l = _hilo(np.ascontiguousarray(W2.T))               # (H, H)
    w3h, w3l = _hilo(np.ascontiguousarray(W3.T))
    w4h, w4l = _hilo(np.ascontiguousarray(W4.T))               # (H, AOUT)
    shared = {
        "w1h": w1h, "w1l": w1l, "w2h": w2h, "w2l": w2l,
        "w3h": w3h, "w3l": w3l, "w4h": w4h, "w4l": w4l,
        "b1": np.ascontiguousarray(b1.reshape(KH, P).T.astype(np.float32)),
        "b2": np.ascontiguousarray(b2.reshape(KH, P).T.astype(np.float32)),
        "b3": np.ascontiguousarray(b3.reshape(KH, P).T.astype(np.float32)),
        "b4f": ((1.0 - 2.0 ** -T) * b4).astype(np.float32).reshape(AOUT, 1),
    }
    in_maps = []
    for i in range(NCORES):
        m = dict(shared)
        m["xh"] = np.ascontiguousarray(xh[:, i * N:(i + 1) * N])
        m["xl"] = np.ascontiguousarray(xl[:, i * N:(i + 1) * N])
        in_maps.append(m)
    return in_maps


def _run(in_maps):
    from concourse.bass_utils import run_bass_kernel_spmd
    if "nc" not in _CACHE:
        _CACHE["nc"] = _build()
    res = run_bass_kernel_spmd(_CACHE["nc"], in_maps, list(range(NCORES)))
    parts = [res.results[i]["v4T"] for i in range(NCORES)]     # each (AOUT, N)
    return np.ascontiguousarray(np.concatenate(parts, axis=1).T).astype(np.float32)


def kernel(x, W1, b1, W2, b2, W3, b3, W4, b4):
    in_maps = _prep_inputs(x, W1, b1, W2, b2, W3, b3, W4, b4)
    return _run(in_maps)
